# revision 34
# baseline (speedup 1.0000x reference)
"""BeatPooling segment-mean kernel for 8 Trainium2 NeuronCores.

Reference computation (per batch row):
    s = clip(bounds[:, 0], 0, T-1); e = max(s+1, min(bounds[:, 1], T))
    mean[m] = sum(frame[s_m:e_m]) / (e_m - s_m)
    out = concat([mean, fourier(pos)], -1) @ W + b         # [M, D]

Sharding: data-parallel over B (one batch row per core).

Algorithm (per core), all matmuls, no gpsimd (ap_gather costs ~30 ns per
index on the Q7 cores -- ~31 us per 1024 indices -- so every
gather-based formulation loses):

  1. Edge matmuls.  For each 128-frame block k, one f32r matmul with a
     host-built stationary operand uslots_k [128, 32]: column 0 is
     all-ones (the block sum), columns 1.. are inclusive prefix masks,
     one per distinct segment-boundary position (s-1 or e-1) falling in
     that block.  The moving operand is the frame tile [128, 512].  The
     PSUM result P'_k[slot, d] holds every within-block prefix the
     output needs.  f32r streams at 1 cycle/row, so the whole 16 MiB
     frame row costs ~64 x 0.3 us of PE time and is never transposed,
     scanned, or cast.
  2. P' tiles are evacuated to SBUF as fp16 (0.05% worst-case error --
     well within the 2e-2 gate).
  3. Combine matmuls.  segT[d, m] = sum_t pv_t^T . G_t, accumulated in
     PSUM over the 16 slot-tiles as they appear.  G_t [128 slots, 512 m]
     (host-built fp16, +-1 one-hots at each segment's e/s boundary
     slots) also absorbs the block-span part: its slot-0 rows carry the
     0/1 band J[k, m] = [K_s(m) <= k < K_e(m)], which multiplies the
     block sums.  So segT accumulates (P_e - P_s + sum of spanned block
     sums) == the full segment sums, transposed, ready for projection.
  4. Projection in fp16 (W1 host-packed), then one scalar_tensor_tensor
     fuses the 1/count scale (per-partition scalar) and the fourier/bias
     term (computed on device from a tiny packed tensor by one more
     matmul per m-tile).

DMA notes: all aux tensors ride in a few large contiguous DMAs (the
original baseline lost ~50 us draining dozens of tiny per-partition
descriptors), and the 16 MiB frame stream alternates between the two
HWDGE rings (sync/scalar) in 2 MiB chunks.
"""

import math

import numpy as np

import concourse.bacc as bacc
import concourse.mybir as mybir
from concourse import bass_utils
from concourse.tile import TileContext

B, T, D, M = 8, 8192, 512, 512
POS_DIM = 32
P = 128
N_CORES = 8
NB = T // P            # 64 blocks of 128 frames
GROUPS = 16            # stream groups (4 blocks = 1 MiB each)
BPG = NB // GROUPS     # blocks per group
DC = D // P            # 4 d-chunks
MC = M // P            # 4 m-chunks

F32 = mybir.dt.float32
F32R = mybir.dt.float32r
F16 = mybir.dt.float16
BF16 = mybir.dt.bfloat16

_CACHED_NC = {}


def _build_nc(S):
    NSLOT = NB * S
    TPB = P // S           # blocks per slot-tile (4 for S=32)
    NT = NB // TPB         # slot-tiles (16)

    nc = bacc.Bacc("TRN2", target_bir_lowering=False, debug=False,
                   num_devices=N_CORES)

    frame = nc.dram_tensor("frame", [T, D], F32R, kind="ExternalInput")
    us_in = nc.dram_tensor("uslots", [P, NSLOT + 4], F32R,
                           kind="ExternalInput")
    g_in = nc.dram_tensor("gmat", [P, NT * M], F16, kind="ExternalInput")
    w1_in = nc.dram_tensor("w1p", [P, DC * D], F16, kind="ExternalInput")
    ffw2_in = nc.dram_tensor("ffw2", [P, D], F32R, kind="ExternalInput")
    out = nc.dram_tensor("out", [M, D], F32, kind="ExternalOutput")

    add = mybir.AluOpType.add
    mult = mybir.AluOpType.mult

    with TileContext(nc, num_cores=N_CORES) as tc:
        with (
            tc.tile_pool(name="const", bufs=1) as const,
            tc.tile_pool(name="staging", bufs=6) as staging,
            tc.tile_pool(name="psum", bufs=4, space="PSUM") as psum,
            tc.tile_pool(name="pacc", bufs=1, space="PSUM") as pacc,
            tc.tile_pool(name="outp", bufs=2) as outp,
        ):
            # ---- long-lived tiles -------------------------------------
            uslots = const.tile([P, NSLOT + 4], F32R, name="uslots")
            gmat = const.tile([P, NT * M], F16, name="gmat")
            pvall = const.tile([P, NT * D], F16, name="pvall")
            w1t = const.tile([P, DC * D], F16, name="w1t")
            ffa = const.tile([64, D], F32R, name="ffa")
            ffb = const.tile([64, D], F32R, name="ffb")
            segsb = const.tile([P, DC * M], F16, name="segsb")
            outall = const.tile([P, MC * D], F32, name="outall")
            biassb = const.tile([P, MC * D], F32, name="biassb")

            recip_v = uslots[:, NSLOT:NSLOT + 4].bitcast(F32)

            # ---- constant DMAs (uslots gates the first edge matmul;
            # gmat/w1 are needed only later and ride mid/late on the
            # sync ring to balance ring bytes) ----
            UH = (NSLOT + 4) // 2
            QW = NT * M // 4
            nc.sync.dma_start(uslots[:, 0:UH], us_in.ap()[:, 0:UH])
            nc.scalar.dma_start(gmat[:, 0:QW], g_in.ap()[:, 0:QW])
            nc.scalar.dma_start(uslots[:, UH:], us_in.ap()[:, UH:])
            nc.scalar.dma_start(ffa[:], ffw2_in.ap()[0:64, :])
            nc.scalar.dma_start(ffb[:], ffw2_in.ap()[64:128, :])

            # segT accumulators, one per d-chunk, live across the stream
            po = [pacc.tile([P, M], F32, name=f"po_{c}", tag=f"po{c}")
                  for c in range(DC)]

            # gmat arrives in just-in-time quarters on the scalar ring
            # so the combine matmuls never wait and the sync ring carries a
            # pure, uninterrupted frame stream

            # ---- stream frame ----------------------------------------
            frame_g = frame.ap().rearrange("(g b p) d -> g p b d", p=P, b=BPG)
            for g in range(GROUPS):
                st = staging.tile([P, BPG * D], F32R, name="stage",
                                  tag="stage")
                if g in (2, 6, 10):
                    q = g // 4 + 1
                    nc.scalar.dma_start(gmat[:, q * QW:(q + 1) * QW],
                                        g_in.ap()[:, q * QW:(q + 1) * QW])
                if g == 12:
                    nc.scalar.dma_start(w1t[:], w1_in.ap())
                nc.sync.dma_start(
                    st[:].rearrange("p (b d) -> p b d", b=BPG), frame_g[g])
                for b in range(BPG):
                    k = g * BPG + b
                    i = k % TPB
                    t = k // TPB
                    pp = psum.tile([S, D], F32, name=f"pp_{k}", tag="ps")
                    nc.tensor.matmul(
                        pp[:],
                        lhsT=uslots[:, k * S:(k + 1) * S],
                        rhs=st[:, b * D:(b + 1) * D],
                        start=True, stop=True,
                    )
                    if i % 2 == 0:
                        nc.vector.tensor_scalar_add(
                            out=pvall[i * S:(i + 1) * S,
                                      t * D:(t + 1) * D],
                            in0=pp[:], scalar1=0.0)
                    else:
                        nc.scalar.copy(
                            pvall[i * S:(i + 1) * S, t * D:(t + 1) * D],
                            pp[:])
                    if i == TPB - 1:
                        # combine: segT[c] += pv_t[:, c]^T @ G_t
                        for c in range(DC):
                            nc.tensor.matmul(
                                po[c][:],
                                lhsT=pvall[:, t * D + c * P:
                                           t * D + (c + 1) * P],
                                rhs=gmat[:, t * M:(t + 1) * M],
                                start=(t == 0), stop=(t == NT - 1),
                                skip_group_check=True,
                            )

            # fourier/bias term: bias[m, j] = ff[m] @ W2 + b (PE slack
            # while the last stream group lands; needed only by the stt)
            for mt in range(MC):
                bps = psum.tile([P, D], F32, name=f"bps_{mt}", tag="ps")
                nc.tensor.matmul(
                    bps[:],
                    lhsT=ffa[:, mt * P:(mt + 1) * P],
                    rhs=ffb[:],
                    start=True, stop=True,
                )
                nc.scalar.copy(biassb[:, mt * D:(mt + 1) * D], bps[:])

            # ---- segT -> SBUF fp16, project, scale, bias --------------
            for c in range(DC):
                nc.vector.tensor_scalar_add(
                    out=segsb[:, c * M:(c + 1) * M], in0=po[c][:],
                    scalar1=0.0)
            for mt in range(MC):
                po2 = psum.tile([P, D], F32, name=f"po2_{mt}", tag="ps")
                for c in range(DC):
                    nc.tensor.matmul(
                        po2[:],
                        lhsT=segsb[:, c * M + mt * P:c * M + (mt + 1) * P],
                        rhs=w1t[:, c * D:(c + 1) * D],
                        start=(c == 0), stop=(c == DC - 1),
                    )
                nc.vector.scalar_tensor_tensor(
                    out=outall[:, mt * D:(mt + 1) * D],
                    in0=po2[:],
                    scalar=recip_v[:, mt:mt + 1],
                    in1=biassb[:, mt * D:(mt + 1) * D],
                    op0=mult,
                    op1=add,
                )
            outv = out.ap().rearrange("(mt p) d -> p mt d", p=P)
            oall = outall[:].rearrange("p (mt d) -> p mt d", mt=MC)
            nc.sync.dma_start(outv[:, 0:2], oall[:, 0:2])
            nc.scalar.dma_start(outv[:, 2:4], oall[:, 2:4])

    nc.compile()
    return nc


def _fourier_features(pos, dim):
    half = dim // 2
    freqs = np.exp(np.linspace(0.0, math.log(1000.0), half))
    ang = pos[..., None] * freqs
    return np.concatenate([np.sin(ang), np.cos(ang)], axis=-1)


def _host_prep(frame_emb, beat_bounds, W, b, S):
    NSLOT = NB * S
    TPB = P // S
    NT = NB // TPB

    s_all = np.clip(beat_bounds[:, :, 0], 0, T - 1).astype(np.int64)
    e_all = np.maximum(
        s_all + 1, np.minimum(beat_bounds[:, :, 1], T)).astype(np.int64)
    recip_all = (1.0 / (e_all - s_all)).astype(np.float32)

    pos = np.clip(np.arange(M, dtype=np.float64) / max(1, M - 1), 0.0, 1.0)
    ff = _fourier_features(pos, POS_DIM)                  # [M, 32]
    # rows 0:64 = [ff^T; ones; pad] (cols = m), rows 64:128 = [W2; b; pad]
    ffw2 = np.zeros((P, D), dtype=np.float32)
    ffw2[0:POS_DIM, :] = ff.T.astype(np.float32)
    ffw2[POS_DIM, :] = 1.0
    ffw2[64:64 + POS_DIM, :] = W[D:D + POS_DIM, :].astype(np.float32)
    ffw2[64 + POS_DIM, :] = b.astype(np.float32)

    w1p = np.ascontiguousarray(
        W[:D, :].astype(np.float16).reshape(DC, P, D)
        .transpose(1, 0, 2).reshape(P, DC * D))

    # U[p, o] = 1.0 if p <= o (inclusive prefix-mask columns)
    U = (np.arange(P)[:, None] <= np.arange(P)[None, :]).astype(np.float32)

    in_maps = []
    for i in range(B):
        s, e = s_all[i], e_all[i]
        allpos = np.concatenate([(s - 1)[s > 0], e - 1])
        uslots = np.zeros((P, NSLOT + 4), dtype=np.float32)
        slotmap = {}
        for k in range(NB):
            offs = np.unique(allpos[(allpos >> 7) == k] & 127)
            if len(offs) > S - 1:
                raise OverflowError(
                    f"block {k}: {len(offs)} boundaries > {S - 1}")
            uslots[:, k * S] = 1.0                         # block-sum slot
            for j, o in enumerate(offs):
                uslots[:, k * S + 1 + j] = U[:, o]
                slotmap[(k, int(o))] = k * S + 1 + j
        uslots[:, NSLOT:NSLOT + 4] = recip_all[i].reshape(MC, P).T

        # G_t[slot, m]: +1 at e-boundary slot, -1 at s-boundary slot,
        # 0/1 block-span band J on the slot-0 rows
        gm = np.zeros((NT, P, M), dtype=np.float32)
        for m in range(M):
            pe = int(e[m]) - 1
            ke = pe >> 7
            sl = slotmap[(ke, pe & 127)]
            gm[sl // P, sl % P, m] += 1.0
            ks = 0
            if s[m] > 0:
                ps = int(s[m]) - 1
                ks = ps >> 7
                sl = slotmap[(ks, ps & 127)]
                gm[sl // P, sl % P, m] -= 1.0
            for k in range(ks, ke):
                sl = k * S
                gm[sl // P, sl % P, m] += 1.0
        gmat = np.ascontiguousarray(
            gm.transpose(1, 0, 2).reshape(P, NT * M)).astype(np.float16)

        in_maps.append({
            "frame": np.ascontiguousarray(frame_emb[i], dtype=np.float32),
            "uslots": uslots,
            "gmat": gmat,
            "w1p": w1p,
            "ffw2": ffw2,
        })
    return in_maps


def get_nc(S=32):
    if S not in _CACHED_NC:
        _CACHED_NC[S] = _build_nc(S)
    return _CACHED_NC[S]


def kernel(frame_emb, beat_bounds, W, b, _trace=False):
    frame_emb = np.asarray(frame_emb)
    beat_bounds = np.asarray(beat_bounds)
    W = np.asarray(W)
    b = np.asarray(b)
    in_maps = None
    for S in (32, 64):
        try:
            in_maps = _host_prep(frame_emb, beat_bounds, W, b, S)
            break
        except OverflowError:
            continue
    if in_maps is None:
        raise RuntimeError("too many segment boundaries per 128-frame block")
    nc = get_nc(S)
    res = bass_utils.run_bass_kernel_spmd(
        nc, in_maps, core_ids=list(range(N_CORES)), trace=_trace)
    out = np.stack([res.results[i]["out"] for i in range(B)], axis=0)
    if _trace:
        kernel.last_results = res
    return out


# revision 42
# speedup vs baseline: 1.1459x; 1.1459x over previous
"""BeatPooling segment-mean kernel for 8 Trainium2 NeuronCores.

Reference computation (per batch row):
    s = clip(bounds[:, 0], 0, T-1); e = max(s+1, min(bounds[:, 1], T))
    mean[m] = sum(frame[s_m:e_m]) / (e_m - s_m)
    out = concat([mean, fourier(pos)], -1) @ W + b         # [M, D]

Sharding: data-parallel over B (one batch row per core).

Algorithm (per core), all matmuls, no gpsimd (ap_gather costs ~30 ns per
index on the Q7 cores -- ~31 us per 1024 indices -- so every
gather-based formulation loses):

  1. Edge matmuls.  For each 128-frame block k, one f32r matmul with a
     host-built stationary operand uslots_k [128, 32]: column 0 is
     all-ones (the block sum), columns 1.. are inclusive prefix masks,
     one per distinct segment-boundary position (s-1 or e-1) falling in
     that block.  The moving operand is the frame tile [128, 512].  The
     PSUM result P'_k[slot, d] holds every within-block prefix the
     output needs.  f32r streams at 1 cycle/row, so the whole 16 MiB
     frame row costs ~64 x 0.3 us of PE time and is never transposed,
     scanned, or cast.
  2. P' tiles are evacuated to SBUF as fp16 (0.05% worst-case error --
     well within the 2e-2 gate).
  3. Combine matmuls.  segT[d, m] = sum_t pv_t^T . G_t, accumulated in
     PSUM over the 16 slot-tiles as they appear.  G_t [128 slots, 512 m]
     (host-built fp16, +-1 one-hots at each segment's e/s boundary
     slots) also absorbs the block-span part: its slot-0 rows carry the
     0/1 band J[k, m] = [K_s(m) <= k < K_e(m)], which multiplies the
     block sums.  So segT accumulates (P_e - P_s + sum of spanned block
     sums) == the full segment sums, transposed, ready for projection.
  4. Projection in fp16 (W1 host-packed), then one scalar_tensor_tensor
     fuses the 1/count scale (per-partition scalar) and the fourier/bias
     term (computed on device from a tiny packed tensor by one more
     matmul per m-tile).

DMA notes: all aux tensors ride in a few large contiguous DMAs (the
original baseline lost ~50 us draining dozens of tiny per-partition
descriptors), and the 16 MiB frame stream alternates between the two
HWDGE rings (sync/scalar) in 2 MiB chunks.
"""

import math

import numpy as np

import concourse.bacc as bacc
import concourse.mybir as mybir
from concourse import bass_utils
from concourse.tile import TileContext

B, T, D, M = 8, 8192, 512, 512
POS_DIM = 32
P = 128
N_CORES = 8
NB = T // P            # 64 blocks of 128 frames
GROUPS = 8             # stream groups (8 blocks = 2 MiB each)
BPG = NB // GROUPS     # blocks per group
DC = D // P            # 4 d-chunks
MC = M // P            # 4 m-chunks

F32 = mybir.dt.float32
F32R = mybir.dt.float32r
F16 = mybir.dt.float16
BF16 = mybir.dt.bfloat16

_CACHED_NC = {}


def _build_nc(S):
    NSLOT = NB * S
    TPB = P // S           # blocks per slot-tile (4 for S=32)
    NT = NB // TPB         # slot-tiles (16)

    nc = bacc.Bacc("TRN2", target_bir_lowering=False, debug=False,
                   num_devices=N_CORES)

    frame = nc.dram_tensor("frame", [T, D], F32R, kind="ExternalInput")
    us_in = nc.dram_tensor("uslots", [P, NSLOT + 4], F32R,
                           kind="ExternalInput")
    g_in = nc.dram_tensor("gmat", [P, NT * M], F16, kind="ExternalInput")
    w1_in = nc.dram_tensor("w1p", [P, DC * D], F16, kind="ExternalInput")
    ffw2_in = nc.dram_tensor("ffw2", [P, D], F32R, kind="ExternalInput")
    out = nc.dram_tensor("out", [M, D], F32, kind="ExternalOutput")

    add = mybir.AluOpType.add
    mult = mybir.AluOpType.mult

    with TileContext(nc, num_cores=N_CORES) as tc:
        with (
            tc.tile_pool(name="const", bufs=1) as const,
            tc.tile_pool(name="staging", bufs=4) as staging,
            tc.tile_pool(name="psum", bufs=4, space="PSUM") as psum,
            tc.tile_pool(name="pacc", bufs=1, space="PSUM") as pacc,
            tc.tile_pool(name="outp", bufs=2) as outp,
        ):
            # ---- long-lived tiles -------------------------------------
            uslots = const.tile([P, NSLOT + 4], F32R, name="uslots")
            gmat = const.tile([P, NT * M], F16, name="gmat")
            pvall = const.tile([P, NT * D], F16, name="pvall")
            w1t = const.tile([P, DC * D], F16, name="w1t")
            ffa = const.tile([64, D], F32R, name="ffa")
            ffb = const.tile([64, D], F32R, name="ffb")
            segsb = const.tile([P, DC * M], F16, name="segsb")
            outall = const.tile([P, MC * D], F32, name="outall")
            biassb = const.tile([P, MC * D], F32, name="biassb")

            recip_v = uslots[:, NSLOT:NSLOT + 4].bitcast(F32)

            # ---- constant DMAs (uslots gates the first edge matmul;
            # gmat/w1 are needed only later and ride mid/late on the
            # sync ring to balance ring bytes) ----
            UQ = (NSLOT + 4) // 4
            QW = NT * M // 4
            nc.sync.dma_start(uslots[:, 0:UQ], us_in.ap()[:, 0:UQ])
            nc.scalar.dma_start(gmat[:, 0:QW], g_in.ap()[:, 0:QW])

            # segT accumulators, one per d-chunk, live across the stream
            po = [pacc.tile([P, M], F32, name=f"po_{c}", tag=f"po{c}")
                  for c in range(DC)]

            # gmat arrives in just-in-time quarters on the scalar ring
            # so the combine matmuls never wait and the sync ring carries a
            # pure, uninterrupted frame stream

            # ---- stream frame ----------------------------------------
            frame_g = frame.ap().rearrange("(g b p) d -> g p b d", p=P, b=BPG)
            for g in range(GROUPS):
                st = staging.tile([P, BPG * D], F32R, name="stage",
                                  tag="stage")
                if g == 1:
                    nc.scalar.dma_start(uslots[:, UQ:2 * UQ],
                                        us_in.ap()[:, UQ:2 * UQ])
                if g in (1, 3, 5):
                    q = g // 2 + 1
                    nc.scalar.dma_start(gmat[:, q * QW:(q + 1) * QW],
                                        g_in.ap()[:, q * QW:(q + 1) * QW])
                if g == 2:
                    nc.scalar.dma_start(uslots[:, 2 * UQ:],
                                        us_in.ap()[:, 2 * UQ:])
                if g == 5:
                    nc.scalar.dma_start(ffa[:], ffw2_in.ap()[0:64, :])
                    nc.scalar.dma_start(ffb[:], ffw2_in.ap()[64:128, :])
                if g == 6:
                    nc.scalar.dma_start(w1t[:], w1_in.ap())
                stv = st[:].rearrange("p (b d) -> p b d", b=BPG)
                if g == 0:
                    nc.sync.dma_start(stv[:, 0:2], frame_g[g][:, 0:2])
                    nc.sync.dma_start(stv[:, 2:4], frame_g[g][:, 2:4])
                    nc.sync.dma_start(stv[:, 4:8], frame_g[g][:, 4:8])
                else:
                    nc.sync.dma_start(stv, frame_g[g])
                for b in range(BPG):
                    k = g * BPG + b
                    i = k % TPB
                    t = k // TPB
                    pp = psum.tile([S, D], F32, name=f"pp_{k}", tag="ps")
                    nc.tensor.matmul(
                        pp[:],
                        lhsT=uslots[:, k * S:(k + 1) * S],
                        rhs=st[:, b * D:(b + 1) * D],
                        start=True, stop=True,
                    )
                    if i % 2 == 0:
                        nc.vector.tensor_scalar_add(
                            out=pvall[i * S:(i + 1) * S,
                                      t * D:(t + 1) * D],
                            in0=pp[:], scalar1=0.0)
                    else:
                        nc.scalar.copy(
                            pvall[i * S:(i + 1) * S, t * D:(t + 1) * D],
                            pp[:])
                    if i == TPB - 1 and t % 4 == 3 and t < NT - 1:
                        # combine, batched 4 tiles at a time and grouped by
                        # chunk so each po[c] accumulation chain runs four
                        # consecutive steps (uninterrupted chains avoid a
                        # per-switch PE overhead)
                        for c in range(DC):
                            for tl in range(t - 3, t + 1):
                                nc.tensor.matmul(
                                    po[c][:],
                                    lhsT=pvall[:, tl * D + c * P:
                                               tl * D + (c + 1) * P],
                                    rhs=gmat[:, tl * M:(tl + 1) * M],
                                    start=(tl == 0), stop=False,
                                )

            # fourier/bias term: bias[m, j] = ff[m] @ W2 + b (PE slack
            # while the last stream group lands; needed only by the stt)
            for mt in range(MC):
                bps = psum.tile([P, D], F32, name=f"bps_{mt}", tag="ps")
                nc.tensor.matmul(
                    bps[:],
                    lhsT=ffa[:, mt * P:(mt + 1) * P],
                    rhs=ffb[:],
                    start=True, stop=True,
                )
                nc.scalar.copy(biassb[:, mt * D:(mt + 1) * D], bps[:])

            # last batch of combines
            for c in range(DC):
                for tl in range(NT - 4, NT):
                    nc.tensor.matmul(
                        po[c][:],
                        lhsT=pvall[:, tl * D + c * P:tl * D + (c + 1) * P],
                        rhs=gmat[:, tl * M:(tl + 1) * M],
                        start=False, stop=(tl == NT - 1),
                    )

            # ---- segT -> SBUF fp16, project, scale, bias --------------
            for c in range(DC):
                nc.vector.tensor_scalar_add(
                    out=segsb[:, c * M:(c + 1) * M], in0=po[c][:],
                    scalar1=0.0)
            for mt in range(MC):
                po2 = psum.tile([P, D], F32, name=f"po2_{mt}", tag="ps")
                for c in range(DC):
                    nc.tensor.matmul(
                        po2[:],
                        lhsT=segsb[:, c * M + mt * P:c * M + (mt + 1) * P],
                        rhs=w1t[:, c * D:(c + 1) * D],
                        start=(c == 0), stop=(c == DC - 1),
                    )
                nc.vector.scalar_tensor_tensor(
                    out=outall[:, mt * D:(mt + 1) * D],
                    in0=po2[:],
                    scalar=recip_v[:, mt:mt + 1],
                    in1=biassb[:, mt * D:(mt + 1) * D],
                    op0=mult,
                    op1=add,
                )
            outv = out.ap().rearrange("(mt p) d -> p mt d", p=P)
            oall = outall[:].rearrange("p (mt d) -> p mt d", mt=MC)
            nc.sync.dma_start(outv[:, 0:2], oall[:, 0:2])
            nc.scalar.dma_start(outv[:, 2:4], oall[:, 2:4])

    nc.compile()
    return nc


def _fourier_features(pos, dim):
    half = dim // 2
    freqs = np.exp(np.linspace(0.0, math.log(1000.0), half))
    ang = pos[..., None] * freqs
    return np.concatenate([np.sin(ang), np.cos(ang)], axis=-1)


def _host_prep(frame_emb, beat_bounds, W, b, S):
    NSLOT = NB * S
    TPB = P // S
    NT = NB // TPB

    s_all = np.clip(beat_bounds[:, :, 0], 0, T - 1).astype(np.int64)
    e_all = np.maximum(
        s_all + 1, np.minimum(beat_bounds[:, :, 1], T)).astype(np.int64)
    recip_all = (1.0 / (e_all - s_all)).astype(np.float32)

    pos = np.clip(np.arange(M, dtype=np.float64) / max(1, M - 1), 0.0, 1.0)
    ff = _fourier_features(pos, POS_DIM)                  # [M, 32]
    # rows 0:64 = [ff^T; ones; pad] (cols = m), rows 64:128 = [W2; b; pad]
    ffw2 = np.zeros((P, D), dtype=np.float32)
    ffw2[0:POS_DIM, :] = ff.T.astype(np.float32)
    ffw2[POS_DIM, :] = 1.0
    ffw2[64:64 + POS_DIM, :] = W[D:D + POS_DIM, :].astype(np.float32)
    ffw2[64 + POS_DIM, :] = b.astype(np.float32)

    w1p = np.ascontiguousarray(
        W[:D, :].astype(np.float16).reshape(DC, P, D)
        .transpose(1, 0, 2).reshape(P, DC * D))

    # U[p, o] = 1.0 if p <= o (inclusive prefix-mask columns)
    U = (np.arange(P)[:, None] <= np.arange(P)[None, :]).astype(np.float32)

    in_maps = []
    for i in range(B):
        s, e = s_all[i], e_all[i]
        allpos = np.concatenate([(s - 1)[s > 0], e - 1])
        uslots = np.zeros((P, NSLOT + 4), dtype=np.float32)
        slotmap = {}
        for k in range(NB):
            offs = np.unique(allpos[(allpos >> 7) == k] & 127)
            if len(offs) > S - 1:
                raise OverflowError(
                    f"block {k}: {len(offs)} boundaries > {S - 1}")
            uslots[:, k * S] = 1.0                         # block-sum slot
            for j, o in enumerate(offs):
                uslots[:, k * S + 1 + j] = U[:, o]
                slotmap[(k, int(o))] = k * S + 1 + j
        uslots[:, NSLOT:NSLOT + 4] = recip_all[i].reshape(MC, P).T

        # G_t[slot, m]: +1 at e-boundary slot, -1 at s-boundary slot,
        # 0/1 block-span band J on the slot-0 rows
        gm = np.zeros((NT, P, M), dtype=np.float32)
        for m in range(M):
            pe = int(e[m]) - 1
            ke = pe >> 7
            sl = slotmap[(ke, pe & 127)]
            gm[sl // P, sl % P, m] += 1.0
            ks = 0
            if s[m] > 0:
                ps = int(s[m]) - 1
                ks = ps >> 7
                sl = slotmap[(ks, ps & 127)]
                gm[sl // P, sl % P, m] -= 1.0
            for k in range(ks, ke):
                sl = k * S
                gm[sl // P, sl % P, m] += 1.0
        gmat = np.ascontiguousarray(
            gm.transpose(1, 0, 2).reshape(P, NT * M)).astype(np.float16)

        in_maps.append({
            "frame": np.ascontiguousarray(frame_emb[i], dtype=np.float32),
            "uslots": uslots,
            "gmat": gmat,
            "w1p": w1p,
            "ffw2": ffw2,
        })
    return in_maps


def get_nc(S=32):
    if S not in _CACHED_NC:
        _CACHED_NC[S] = _build_nc(S)
    return _CACHED_NC[S]


def kernel(frame_emb, beat_bounds, W, b, _trace=False):
    frame_emb = np.asarray(frame_emb)
    beat_bounds = np.asarray(beat_bounds)
    W = np.asarray(W)
    b = np.asarray(b)
    in_maps = None
    for S in (32, 64):
        try:
            in_maps = _host_prep(frame_emb, beat_bounds, W, b, S)
            break
        except OverflowError:
            continue
    if in_maps is None:
        raise RuntimeError("too many segment boundaries per 128-frame block")
    nc = get_nc(S)
    res = bass_utils.run_bass_kernel_spmd(
        nc, in_maps, core_ids=list(range(N_CORES)), trace=_trace)
    out = np.stack([res.results[i]["out"] for i in range(B)], axis=0)
    if _trace:
        kernel.last_results = res
    return out
